# revision 1
# baseline (speedup 1.0000x reference)
"""Trainium2 Bass kernel for nn_Attention_13700945674736 (sparse local-window attention).

Strategy (8 NeuronCores, data-parallel over batch, 4 samples/core):
  - Permute the sequence axis s = 64*i + j  ->  s' = 16*j + i (image transpose).
    The 7x11 local window becomes a 1-D band |ds'| <= 83, so each 128-query
    tile only attends to 3 aligned 128-key chunks (384 keys) instead of 1024.
  - Heads are padded to 64-partition slots (host-padded weights) so every
    engine access pattern starts at a 32-aligned partition.
  - All matmul operands are bf16 (PE streams bf16 4x faster than fp32);
    PSUM accumulation and the softmax reductions stay fp32.
  - attnT[k, q] exact-band tiles (per-chunk q-windows of width <= 296); exp on
    ScalarE with fused 1/sqrt(d) scale; binary window mask applied
    multiplicatively on VectorE (bf16 2x); @V uses lhsT=[V|0|ones|0] so softmax
    denominators land at partitions 64:112 of the same PSUM tile; the bias is
    folded into the projection via a constant-1 row.
  - Softmax skips max-subtraction (|logit| small; exact in fp32).
  - q-tiles of the @V accumulation alternate PSUM banks because start=True
    zeroes the whole bank's has_written bits.
"""

import sys

sys.path.insert(0, "/opt/trn_rl_repo")

import numpy as np

import concourse.bass as bass
from concourse import bacc
import concourse.mybir as mybir
import concourse.tile as tile
from concourse.bass_utils import run_bass_kernel_spmd

# ---------------------------------------------------------------- constants
B, S, C = 32, 1024, 384
H, D = 8, 48
HI, WI = 16, 64
N_CORES = 8
BL = B // N_CORES  # samples per core
SCALE = float(D) ** -0.5
F32 = mybir.dt.float32
BF16 = mybir.dt.bfloat16

# precision of the softmax-weights path (expT / m01 / vv).
PD = BF16

# s' = 16*j + i  <->  s = 64*i + j ;  PERM[s'] = s
_sp = np.arange(S)
PERM = (_sp % HI) * WI + (_sp // HI)

NQT = S // 128  # 8 query tiles (and key chunks)
WPADQ = 64 * H  # padded Q (and K) section width: 512


# exact per-chunk bands: key-chunk c attends to queries [QLO[c], QHI[c])
# (halfwidth 84 >= true window 83; even offsets keep bf16 APs 4B-aligned)
QLO = [max(0, 128 * c - 84) for c in range(NQT)]
QHI = [min(S, 128 * c + 212) for c in range(NQT)]
WC = [QHI[c] - QLO[c] for c in range(NQT)]
OFFC = list(np.cumsum([0] + WC[:-1]))
BAND_W = sum(WC)  # 2200
# chunk groups per PSUM attn tile (<= 508 f32 -> ONE bank, double-buffered;
# the freed banks let the @V accumulator double-buffer across heads)
CH_GROUPS = [(0, 1), (2,), (3,), (4,), (5,), (6, 7)]
GRP_BASE = [OFFC[g[0]] for g in CH_GROUPS]
QTILE_W = max(OFFC[g[-1]] + WC[g[-1]] - OFFC[g[0]] for g in CH_GROUPS)

# ---------------------------------------------------------------- bass program
_CACHE = {}
_LAST_IN_MAPS = None


def _build():
    if "nc" in _CACHE:
        return _CACHE["nc"]

    nc = bacc.Bacc(None, target_bir_lowering=False)
    xT_d = nc.declare_dram_parameter("xT", [BL, C, S], BF16, isOutput=False)
    wq_d = nc.declare_dram_parameter("wq_pad", [C, 2 * WPADQ + C], BF16, isOutput=False)
    wp_d = nc.declare_dram_parameter("wp_pad", [4, 128, C], BF16, isOutput=False)
    ones_d = nc.declare_dram_parameter("ones_row", [1, S], BF16, isOutput=False)
    m_d = nc.declare_dram_parameter("m01", [128, BAND_W], PD, isOutput=False)
    out_d = nc.declare_dram_parameter("out", [BL, S, C], F32, isOutput=True)

    WQW = 2 * WPADQ + C  # 1408

    with tile.TileContext(nc) as tc:
        with (
            tc.tile_pool(name="singles", bufs=1) as singles,
            tc.tile_pool(name="xt_pool", bufs=3) as xt_pool,
            tc.tile_pool(name="out_pool", bufs=4) as out_pool,
            tc.tile_pool(name="ps_small", bufs=2, space="PSUM") as ps_small,
            tc.tile_pool(name="ps_attn", bufs=4, space="PSUM") as ps_attn,
            tc.tile_pool(name="ps_outv", bufs=1, space="PSUM") as ps_outv,
        ):
            # ---- constants
            w_sb = singles.tile([128, 3, WQW], BF16)
            nc.sync.dma_start(w_sb[:, :, :], wq_d.rearrange("(c p) w -> p c w", p=128))
            wp_sb = singles.tile([128, 4, C], BF16)
            nc.gpsimd.dma_start(wp_sb[:, :, :], wp_d.rearrange("f p c -> p f c"))
            m_sb = singles.tile([128, BAND_W], PD)
            nc.gpsimd.dma_start(m_sb, m_d[:, :])

            # ---- per-sample tiles, double-buffered for cross-sample overlap
            qTs, kTs, vvs, aoTs, expTs, dens = [], [], [], [], [], []
            for i in range(2):
                qTs.append(singles.tile([128, 4, S], BF16, name=f"qT{i}"))
                kTs.append(singles.tile([128, 4, S], BF16, name=f"kT{i}"))
                vvs.append(singles.tile([128, NQT, H, 128], PD, name=f"vv{i}"))
                aoTs.append(singles.tile([128, 4, S], BF16, name=f"aoT{i}"))
                expTs.append(singles.tile([128, BAND_W], PD, name=f"expT{i}"))
                dens.append(singles.tile([48, S], F32, name=f"den{i}"))
            for vv in vvs:
                nc.gpsimd.memset(vv[:, :, :, D : D + 16], 0.0)
                nc.gpsimd.memset(vv[:, :, :, D + 16 : 112], 1.0)
                nc.gpsimd.memset(vv[:, :, :, 112:128], 0.0)
            for aoT in aoTs:
                # zero dead rows (48:64, 112:128); starts must be 32-aligned so
                # cover 32:64 / 96:128 — live rows are rewritten by the divides.
                nc.gpsimd.memset(aoT[32:64, :, :], 0.0)
                nc.gpsimd.memset(aoT[96:128, :, :], 0.0)
                # constant-1 row: proj picks up b_proj from wp_pad[0][48]
                # (DMA: engine APs cannot start at partition 48)
                nc.gpsimd.dma_start(aoT[48:49, 0, :], ones_d[:, :])

            for b in range(BL):
                qT, kT, vv, aoT = qTs[b % 2], kTs[b % 2], vvs[b % 2], aoTs[b % 2]
                # ---------------- load x^T (3 chunks of [128, 1024])
                xt = xt_pool.tile([128, 3, S], BF16)
                nc.sync.dma_start(
                    xt[:, :, :], xT_d[b].rearrange("(c p) s -> p c s", p=128)
                )

                # ---------------- QKV projection
                # Q/K: padded head-pair tiles -> single full-tile evacuations
                for qk in range(2):
                    dst = qT if qk == 0 else kT
                    for pair in range(4):
                        ncol = qk * WPADQ + pair * 128
                        for half in range(2):
                            ps = ps_small.tile([128, 512], F32, tag="mm")
                            for ci in range(3):
                                nc.tensor.matmul(
                                    ps[:, :],
                                    w_sb[:, ci, ncol : ncol + 128],
                                    xt[:, ci, half * 512 : (half + 1) * 512],
                                    start=(ci == 0),
                                    stop=(ci == 2),
                                )
                            seg_dst = dst[:, pair, half * 512 : (half + 1) * 512]
                            nc.scalar.copy(seg_dst, ps[:, :])

                # V: natural layout -> vv (cast to PD)
                for st in range(NQT):
                    psv = ps_small.tile([128, C], F32, tag="mm")
                    for ci in range(3):
                        nc.tensor.matmul(
                            psv[:, :],
                            xt[:, ci, st * 128 : (st + 1) * 128],
                            w_sb[:, ci, 2 * WPADQ : 2 * WPADQ + C],
                            start=(ci == 0),
                            stop=(ci == 2),
                        )
                    nc.vector.tensor_copy(
                        vv[:, st, :, 0:D],
                        psv[:, :].rearrange("p (h d) -> p h d", h=H),
                    )

                # ---------------- attention, head by head
                for h in range(H):
                    pair, sub = divmod(h, 2)
                    p0 = sub * 64
                    expT = expTs[h % 2]
                    den_sb = dens[h % 2]
                    for gi, grp in enumerate(CH_GROUPS):
                        gbase = GRP_BASE[gi]
                        gw = OFFC[grp[-1]] + WC[grp[-1]] - gbase
                        pat = ps_attn.tile([128, QTILE_W], F32, tag="attn")
                        for c in grp:
                            # one matmul per PSUM-bank-aligned piece of the band
                            lo = OFFC[c] - gbase
                            hi = lo + WC[c]
                            a = lo
                            while a < hi:
                                b2 = min(hi, (a // 512 + 1) * 512)
                                nc.tensor.matmul(
                                    pat[:, a:b2],
                                    kT[p0 : p0 + D, pair, c * 128 : (c + 1) * 128],
                                    qT[p0 : p0 + D, pair, QLO[c] + (a - lo) : QLO[c] + (b2 - lo)],
                                    start=True,
                                    stop=True,
                                )
                                a = b2
                        nc.scalar.activation(
                            expT[:, gbase : gbase + gw],
                            pat[:, 0:gw],
                            mybir.ActivationFunctionType.Exp,
                            scale=SCALE,
                        )
                        # binary window mask (DVE, bf16 2x); per group so @V
                        # can start before the last group's exp
                        nc.vector.tensor_tensor(
                            expT[:, gbase : gbase + gw],
                            expT[:, gbase : gbase + gw],
                            m_sb[:, gbase : gbase + gw],
                            mybir.AluOpType.mult,
                        )
                    # @V with ones rows at 64:112 -> denominators.
                    # start=True zeroes the whole PSUM bank's has_written bits,
                    # so concurrently-pending accumulation groups must not share
                    # a bank: q-tile t lives at col (t%2)*512 + (t//2)*128 (even
                    # tiles in bank 0, odd in bank 1; only adjacent tiles are
                    # pending simultaneously).
                    po = ps_outv.tile([128, S], F32, tag="outv")
                    for c in range(NQT):
                        lhsT = vv[:, c, h, :]
                        for t in range(max(c - 1, 0), min(c + 2, NQT)):
                            pc = (t % 2) * 512 + (t // 2) * 128
                            qs = max(128 * t, QLO[c])
                            qe = min(128 * t + 128, QHI[c])
                            nc.tensor.matmul(
                                po[:, pc + (qs - 128 * t) : pc + (qe - 128 * t)],
                                lhsT,
                                expT[:, OFFC[c] + (qs - QLO[c]) : OFFC[c] + (qe - QLO[c])],
                                start=(c == max(t - 1, 0)),
                                stop=(c == min(t + 1, NQT - 1)),
                            )
                    # normalize: TT-divide is not a valid DVE op, so reciprocal
                    # (PSUM->SBUF) then multiply (one PSUM operand is legal).
                    # Read po back in q-order via a free-dim permuting AP.
                    po_q = po[:, :].rearrange("p (o a u) -> p a o u", o=2, a=4, u=128)
                    den_v = den_sb[:, :].rearrange("p (a o u) -> p a o u", a=4, o=2, u=128)
                    ao_v = aoT[p0 : p0 + D, pair, :].rearrange(
                        "p (a o u) -> p a o u", a=4, o=2, u=128
                    )
                    nc.vector.reciprocal(den_v, po_q[64 : 64 + D])
                    nc.vector.tensor_tensor(
                        ao_v, po_q[0:D], den_v, mybir.AluOpType.mult
                    )

                # ---------------- output projection (+bias), store
                for st in range(NQT):
                    psp = ps_attn.tile([128, C], F32, tag="attn")
                    for p in range(4):
                        nc.tensor.matmul(
                            psp[:, :],
                            aoT[:, p, st * 128 : (st + 1) * 128],
                            wp_sb[:, p, :],
                            start=(p == 0),
                            stop=(p == 3),
                        )
                    ot = out_pool.tile([128, C], F32)
                    if st % 2 == 0:
                        nc.scalar.copy(ot[:, :], psp[:, :])
                    else:
                        nc.vector.tensor_copy(ot[:, :], psp[:, :])
                    nc.scalar.dma_start(out_d[b, st * 128 : (st + 1) * 128, :], ot[:, :])

    nc.finalize()
    _CACHE["nc"] = nc
    return nc


# ---------------------------------------------------------------- host wrapper
def _np_bf16(a):
    import ml_dtypes

    return np.asarray(a, dtype=ml_dtypes.bfloat16)


def _build_m01(mask):
    """[128, BAND_W] banded 0/1 mask in exact-band layout (rows = key within
    chunk c, cols = q in [QLO[c], QHI[c]))."""
    mp = np.asarray(mask)[np.ix_(PERM, PERM)]
    good = np.isfinite(mp) & (mp == 0.0)
    m01 = np.zeros((128, BAND_W), np.float32)
    covered = 0
    for c in range(NQT):
        blk = good[QLO[c] : QHI[c], c * 128 : (c + 1) * 128]  # [q, k]
        m01[:, OFFC[c] : OFFC[c] + WC[c]] = blk.T.astype(np.float32)
        covered += int(blk.sum())
    assert covered == int(good.sum()), "mask not covered by band layout"
    return m01


def _pad_wqkv(w_qkv):
    """[384, 1152] -> [384, 1408]: Q/K head h at cols h*64..h*64+48 (zero pad),
    V kept natural at cols 1024:1408."""
    out = np.zeros((C, 2 * WPADQ + C), np.float32)
    for sec in range(2):  # Q, K
        for h in range(H):
            out[:, sec * WPADQ + h * 64 : sec * WPADQ + h * 64 + D] = w_qkv[
                :, sec * C + h * D : sec * C + (h + 1) * D
            ]
    out[:, 2 * WPADQ :] = w_qkv[:, 2 * C :]
    return out


def _pad_wproj(w_proj, b_proj):
    """[384, 384] -> [4, 128, 384]: pair p rows 0:48 = head 2p, 64:112 = head 2p+1.
    Row 48 of pair 0 carries b_proj (matched by the constant-1 row in aoT)."""
    out = np.zeros((4, 128, C), np.float32)
    for p in range(4):
        out[p, 0:D] = w_proj[(2 * p) * D : (2 * p + 1) * D]
        out[p, 64 : 64 + D] = w_proj[(2 * p + 1) * D : (2 * p + 2) * D]
    out[0, D] = b_proj
    return out


def kernel(x, w_qkv, w_proj, b_proj, mask):
    global _LAST_IN_MAPS
    x = np.asarray(x, np.float32)
    w_qkv = np.asarray(w_qkv, np.float32)
    w_proj = np.asarray(w_proj, np.float32)
    b_proj = np.asarray(b_proj, np.float32)

    nc = _build()

    xT = _np_bf16(np.ascontiguousarray(x[:, PERM, :].transpose(0, 2, 1)))  # [B, C, S']
    wq_pad = _np_bf16(_pad_wqkv(w_qkv))
    wp_pad = _np_bf16(_pad_wproj(w_proj, b_proj))
    ones_row = _np_bf16(np.ones((1, S), np.float32))
    m01 = _build_m01(mask)
    if PD == BF16:
        m01 = _np_bf16(m01)

    in_maps = [
        {
            "xT": xT[c * BL : (c + 1) * BL],
            "wq_pad": wq_pad,
            "wp_pad": wp_pad,
            "ones_row": ones_row,
            "m01": m01,
        }
        for c in range(N_CORES)
    ]
    _LAST_IN_MAPS = in_maps
    res = run_bass_kernel_spmd(nc, in_maps, list(range(N_CORES)))
    out_p = np.concatenate([res.results[c]["out"] for c in range(N_CORES)], axis=0)
    out = np.empty_like(out_p)
    out[:, PERM, :] = out_p
    return out

